# revision 17
# baseline (speedup 1.0000x reference)
"""Trainium2 Bass kernel for nn_EnhancedFinancialGAT.

Mathematical collapse: the reference broadcasts each batch item's feature
vector to all N=2000 graph nodes, so every node starts identical. A GAT
layer on identical node features returns, for every node, the attention-
weighted average of identical projected vectors -- and per-dst softmax
weights sum to exactly 1 in f32 (denom + 1e-16 == denom), so each layer
reduces to relu(h @ W.T + b). Every node stays identical through all 3
layers, and the company-node gather picks that shared vector. The whole
model is therefore an MLP:

  h = relu(x @ W_in.T + b_in)
  h = relu(h @ gat_W[l].T + gat_b[l])   for l in 0..2
  fused = relu(concat([h, emb[company_indices]]) @ W_fuse.T + b_fuse)
  price = W_p3 @ relu(W_p2 @ relu(W_p1 @ fused + b_p1) + b_p2) + b_p3
  direction = sigmoid(same with d-weights)

Verified numerically: collapsed-vs-full relative error ~2e-7 (pure f32
rounding noise of the softmax-weighted sums). The whole kernel is exact
f32: end-to-end relative error vs the reference is ~6e-7.

Sharding: data-parallel over batch (64 rows -> 8 rows/core). Weights are
replicated, pre-transposed on host into per-chunk contiguous DRAM
tensors (activations-transposed layout [feature, batch], so no on-device
transposes are needed anywhere).

Raw Bass program (no TileContext, no Block): everything lives in the
single entry basic block -- no branches (so no IRAM I$-miss stalls) and
no all-engine barrier waves beyond the framework's fixed preamble.
Loads are spread over the three DMA-capable engine queues (sync and
scalar are HWDGE, gpsimd is SWDGE) ordered by when their consumer needs
them, so the fp32 matmul chain starts as soon as chunk0 lands and rarely
waits on DMA again. The sigmoid ACT table set is preloaded via a dummy
activation while weights stream in. PSUM: 4 rotating [128,8] banks for
the wide groups (WAR is covered by the RAW waits since DVE is in-order)
+ 4 dedicated banks for the head tails = exactly 8 banks.
"""

import numpy as np

USE_F16 = False  # fp32 is exact (6e-7); f16 lands ~1e-3 (PE computes 16-bit at bf16 precision)

B = 64
N_CORES = 8
BPC = B // N_CORES  # batch rows per core

# -------- packed [128, COLS] layout (column offsets) --------
OFF_ACTS = 0                       # 16: xT 0:8, embT 8:16 (rows 0:64, dup to 64:128)
OFF_BIAS = 16                      # 16: see bias map below
OFF_WIN = 32                       # 128: rows 0:64 -> M[0:128], rows 64:128 -> M[128:256]
OFF_GAT = 176                      # 6 blocks of 256: block (l,k) at (2l+k)*256
OFF_FUSE = OFF_GAT + 6 * 256       # k0 [128,256], k1 [128,256]
OFF_FUSE2 = OFF_FUSE + 512         # k2 row-split, 128 cols
OFF_P1 = OFF_FUSE2 + 128           # k0 cols 0:128, k1 cols 128:256
OFF_D1 = OFF_P1 + 256
OFF_P2 = OFF_D1 + 256              # [128, 64]
OFF_D2 = OFF_P2 + 64
OFF_P3 = OFF_D2 + 64               # rows 0:64, 1 col
OFF_D3 = OFF_P3 + 1
COLS = OFF_D3 + 1                  # 2994

# bias columns (OFF_BIAS + j):
#  0,1: b_in | 2..7: gat_b (l,m) | 8,9: b_fuse | 10: b_p1 | 11: b_d1
#  12: b_p2 (rows 0:64) | 13: b_d2 | 14 row0: b_p3 | 15 row0: b_d3

# chunk boundaries: c0 = acts+bias+w_in, c1/c2/c3 = gat layers,
# c4 = fuse, c5 = heads
CHUNKS = [0, OFF_GAT, OFF_GAT + 512, OFF_GAT + 1024, OFF_FUSE, OFF_P1, COLS]

_CACHE = {}


def _build_nc():
    from contextlib import ExitStack

    import concourse.bass as bass
    import concourse.mybir as mybir

    f32 = mybir.dt.float32
    f16 = mybir.dt.float16 if USE_F16 else mybir.dt.float32
    ADD = mybir.AluOpType.add
    MAX = mybir.AluOpType.max

    nc = bass.Bass("TRN2", debug=False, num_devices=N_CORES)
    # one contiguous DRAM tensor per chunk: strided slices of a single
    # wide tensor DMA slowly; contiguous blocks run at queue line rate
    packs = [
        nc.declare_dram_parameter(f"pack{i}", [128, c1 - c0], f16, isOutput=False)
        for i, (c0, c1) in enumerate(zip(CHUNKS[:-1], CHUNKS[1:]))
    ]
    out_d = nc.declare_dram_parameter("out", [2, BPC], f32, isOutput=True)

    ctx = ExitStack()
    with ctx:
        sb = lambda nm, shape, dt: ctx.enter_context(nc.sbuf_tensor(nm, shape, dt))
        psb = lambda nm, shape: ctx.enter_context(nc.psum_tensor(nm, shape, f32))
        W = sb("W", [128, COLS], f16)
        h0 = [sb(f"h0_{i}", [128, BPC], f16) for i in range(2)]
        g1 = [sb(f"g1_{i}", [128, BPC], f16) for i in range(2)]
        g2 = [sb(f"g2_{i}", [128, BPC], f16) for i in range(2)]
        g3 = [sb(f"g3_{i}", [128, BPC], f16) for i in range(2)]
        fu = [sb(f"fu_{i}", [128, BPC], f16) for i in range(2)]
        a1p = sb("a1p", [128, BPC], f16)
        a1d = sb("a1d", [128, BPC], f16)
        a2p = sb("a2p", [64, BPC], f16)
        a2d = sb("a2d", [64, BPC], f16)
        price = sb("price", [1, BPC], f32)
        dirn = sb("dirn", [1, BPC], f32)
        scratch = sb("scratch", [1, BPC], f32)
        A = [psb(f"A{i}", [128, BPC]) for i in range(4)]
        Pp2 = psb("Pp2", [64, BPC])
        Pd2 = psb("Pd2", [64, BPC])
        Pp3 = psb("Pp3", [1, BPC])
        Pd3 = psb("Pd3", [1, BPC])

        csem = [ctx.enter_context(nc.semaphore(f"c{i}")) for i in range(6)]
        pe_sem = ctx.enter_context(nc.semaphore("pe"))
        dve_sem = ctx.enter_context(nc.semaphore("dve"))
        store_sem = ctx.enter_context(nc.semaphore("store"))
        all_sems = csem + [pe_sem, dve_sem, store_sem]

        # ---- loads: three parallel engine queues, ordered by need time.
        # Per-queue transfers serialize, so each stream is ordered by its
        # consumers: sync c0 (input layer) then gat1; scalar gat0 (+ ACT
        # table preload after) then fuse; gpsimd gat2 then heads.
        def load(eng, i):
            c0, c1 = CHUNKS[i], CHUNKS[i + 1]
            eng.dma_start(out=W[:, c0:c1], in_=packs[i][:]).then_inc(csem[i], 16)

        # SWDGE (gpsimd) follows the good size/BW curve (~135GB/s at 256KB)
        # and carries the bulk in need-order; the two slow HWDGE rings
        # (sync/scalar, ~50GB/s) each take one chunk off the SWDGE stream.
        load(nc.gpsimd, 0)
        load(nc.gpsimd, 1)
        load(nc.scalar, 2)
        # preload the sigmoid table set early (off the critical path)
        nc.scalar.activation(
            scratch[:], scratch[:], mybir.ActivationFunctionType.Sigmoid
        )
        load(nc.gpsimd, 3)
        load(nc.gpsimd, 4)
        load(nc.sync, 5)

        def bias(j, r1=128, r0=0):
            return W[r0:r1, OFF_BIAS + j : OFF_BIAS + j + 1]

        # ---- PE program -------------------------------------------------
        pe = nc.tensor
        peng = nc.engines[mybir.EngineType.PE]

        def mm(out, lhsT, rhs, start, stop, inc=False):
            m = pe.matmul(out, lhsT, rhs, start=start, stop=stop)
            if inc:
                m.then_inc(pe_sem, 1)

        peng.wait_ge(csem[0], 16)
        for m in range(2):
            r = slice(64 * m, 64 * (m + 1))
            mm(A[m][:], W[r, OFF_WIN : OFF_WIN + 128], W[r, 0:BPC],
               True, True, inc=True)
        hh = h0
        for l in range(3):
            out = (g1, g2, g3)[l]
            c0 = OFF_GAT + (2 * l) * 256
            c1 = OFF_GAT + (2 * l + 1) * 256
            bank = [A[(2 + 2 * l) % 4], A[(3 + 2 * l) % 4]]
            peng.wait_ge(csem[1 + l], 16)
            peng.wait_ge(dve_sem, 2 * l + 1)          # hh[0] ready
            for m in range(2):
                mm(bank[m][:], W[:, c0 + m * 128 : c0 + m * 128 + 128],
                   hh[0][:], True, False)
            peng.wait_ge(dve_sem, 2 * l + 2)          # hh[1] ready
            for m in range(2):
                mm(bank[m][:], W[:, c1 + m * 128 : c1 + m * 128 + 128],
                   hh[1][:], False, True, inc=True)
            hh = out
        peng.wait_ge(csem[4], 16)
        peng.wait_ge(dve_sem, 7)
        for m in range(2):
            mm(A[m][:], W[:, OFF_FUSE + m * 128 : OFF_FUSE + m * 128 + 128],
               hh[0][:], True, False)
        peng.wait_ge(dve_sem, 8)
        for m in range(2):
            mm(A[m][:], W[:, OFF_FUSE + 256 + m * 128 : OFF_FUSE + 256 + m * 128 + 128],
               hh[1][:], False, False)
        for m in range(2):
            r = slice(64 * m, 64 * (m + 1))
            mm(A[m][:], W[r, OFF_FUSE2 : OFF_FUSE2 + 128], W[r, BPC : 2 * BPC],
               False, True, inc=True)
        peng.wait_ge(csem[5], 16)
        peng.wait_ge(dve_sem, 9)                      # fu[0]
        mm(A[2][:], W[:, OFF_P1 : OFF_P1 + 128], fu[0][:], True, False)
        mm(A[3][:], W[:, OFF_D1 : OFF_D1 + 128], fu[0][:], True, False)
        peng.wait_ge(dve_sem, 10)                     # fu[1]
        mm(A[2][:], W[:, OFF_P1 + 128 : OFF_P1 + 256], fu[1][:], False, True, inc=True)
        mm(A[3][:], W[:, OFF_D1 + 128 : OFF_D1 + 256], fu[1][:], False, True, inc=True)
        peng.wait_ge(dve_sem, 11)
        mm(Pp2[:], W[:, OFF_P2 : OFF_P2 + 64], a1p[:], True, True, inc=True)
        peng.wait_ge(dve_sem, 12)
        mm(Pd2[:], W[:, OFF_D2 : OFF_D2 + 64], a1d[:], True, True, inc=True)
        peng.wait_ge(dve_sem, 13)
        mm(Pp3[:], W[0:64, OFF_P3 : OFF_P3 + 1], a2p[:], True, True, inc=True)
        peng.wait_ge(dve_sem, 14)
        mm(Pd3[:], W[0:64, OFF_D3 : OFF_D3 + 1], a2d[:], True, True, inc=True)

        # ---- DVE program ------------------------------------------------
        veng = nc.engines[mybir.EngineType.DVE]

        def rb(out, psum, j, pe_need, r1=128):
            veng.wait_ge(pe_sem, pe_need)
            nc.vector.tensor_scalar(
                out, psum, bias(j, r1), 0.0, ADD, MAX
            ).then_inc(dve_sem, 1)

        rb(h0[0][:], A[0][:], 0, 1)
        rb(h0[1][:], A[1][:], 1, 2)
        for l, out in enumerate((g1, g2, g3)):
            for m in range(2):
                rb(out[m][:], A[(2 + 2 * l + m) % 4][:], 2 + 2 * l + m,
                   3 + 2 * l + m)
        rb(fu[0][:], A[0][:], 8, 9)
        rb(fu[1][:], A[1][:], 9, 10)
        rb(a1p[:], A[2][:], 10, 11)
        rb(a1d[:], A[3][:], 11, 12)
        rb(a2p[:], Pp2[:], 12, 13, r1=64)
        rb(a2d[:], Pd2[:], 13, 14, r1=64)
        veng.wait_ge(pe_sem, 15)
        nc.vector.tensor_scalar(
            price[:], Pp3[:], bias(14, 1), None, ADD
        ).then_inc(dve_sem, 1)

        # ---- ACT: real sigmoid + dir store (scalar is HWDGE-capable) ----
        aeng = nc.engines[mybir.EngineType.Activation]
        aeng.wait_ge(pe_sem, 16)
        nc.scalar.activation(
            dirn[:], Pd3[:], mybir.ActivationFunctionType.Sigmoid,
            bias=bias(15, 1),
        )
        nc.scalar.dma_start(out=out_d[1:2, :], in_=dirn[:]).then_inc(store_sem, 16)

        # ---- sync: price store ------------------------------------------
        seng = nc.engines[mybir.EngineType.SP]
        seng.wait_ge(dve_sem, 15)
        nc.sync.dma_start(out=out_d[0:1, :], in_=price[:]).then_inc(store_sem, 16)

        # ---- gpsimd: sole final waiter; clear sems for re-execution -----
        geng = nc.engines[mybir.EngineType.Pool]
        geng.wait_ge(store_sem, 32)
        nums = sorted(s.num for s in all_sems)
        lo, hi = nums[0], nums[-1]
        assert nums == list(range(lo, hi + 1)), nums
        nc.gpsimd.dma_reset(range(lo, hi + 1))
        nc.gpsimd.sem_clear(range(lo, hi + 1))

    return nc


def _pack_host(inputs):
    f32 = lambda k: np.ascontiguousarray(np.asarray(inputs[k], dtype=np.float32))
    W_in, b_in = f32("W_in"), f32("b_in")
    gat_W, gat_b = f32("gat_W"), f32("gat_b")
    W_fuse, b_fuse = f32("W_fuse"), f32("b_fuse")
    W_p1, b_p1 = f32("W_p1"), f32("b_p1")
    W_p2, b_p2 = f32("W_p2"), f32("b_p2")
    W_p3, b_p3 = f32("W_p3"), f32("b_p3")
    W_d1, b_d1 = f32("W_d1"), f32("b_d1")
    W_d2, b_d2 = f32("W_d2"), f32("b_d2")
    W_d3, b_d3 = f32("W_d3"), f32("b_d3")

    np16 = np.float16 if USE_F16 else np.float32
    pk = np.zeros((128, COLS), np16)

    bias = pk[:, OFF_BIAS : OFF_BIAS + 16]
    bias[:, 0], bias[:, 1] = b_in[:128], b_in[128:]
    for l in range(3):
        for m in range(2):
            bias[:, 2 + 2 * l + m] = gat_b[l, 128 * m : 128 * (m + 1)]
    bias[:, 8], bias[:, 9] = b_fuse[:128], b_fuse[128:]
    bias[:, 10], bias[:, 11] = b_p1, b_d1
    bias[:64, 12], bias[:64, 13] = b_p2, b_d2
    bias[0, 14], bias[0, 15] = b_p3[0], b_d3[0]

    WinT = W_in.T.astype(np16)  # [64, 256]
    pk[0:64, OFF_WIN : OFF_WIN + 128] = WinT[:, 0:128]
    pk[64:128, OFF_WIN : OFF_WIN + 128] = WinT[:, 128:256]
    for l in range(3):
        GT = gat_W[l].T.astype(np16)  # [256, 256]
        for k in range(2):
            c = OFF_GAT + (2 * l + k) * 256
            pk[:, c : c + 256] = GT[128 * k : 128 * (k + 1), :]
    FT = W_fuse.T.astype(np16)  # [320, 256]
    pk[:, OFF_FUSE : OFF_FUSE + 256] = FT[0:128]
    pk[:, OFF_FUSE + 256 : OFF_FUSE + 512] = FT[128:256]
    pk[0:64, OFF_FUSE2 : OFF_FUSE2 + 128] = FT[256:320, 0:128]
    pk[64:128, OFF_FUSE2 : OFF_FUSE2 + 128] = FT[256:320, 128:256]
    for W1, off in ((W_p1, OFF_P1), (W_d1, OFF_D1)):
        T = W1.T.astype(np16)  # [256, 128]
        pk[:, off : off + 128] = T[0:128]
        pk[:, off + 128 : off + 256] = T[128:256]
    pk[:, OFF_P2 : OFF_P2 + 64] = W_p2.T.astype(np16)
    pk[:, OFF_D2 : OFF_D2 + 64] = W_d2.T.astype(np16)
    pk[0:64, OFF_P3] = W_p3[0].astype(np16)
    pk[0:64, OFF_D3] = W_d3[0].astype(np16)
    return pk


def _build_in_maps(inputs):
    x = np.asarray(inputs["x"], dtype=np.float32)
    ci = np.asarray(inputs["company_indices"]).astype(np.int64)
    emb = np.asarray(inputs["emb"], dtype=np.float32)
    comp_emb = emb[ci]  # [B, 64]

    base = _pack_host(inputs)
    in_maps = []
    for c in range(N_CORES):
        pk = base.copy()
        rows = slice(c * BPC, (c + 1) * BPC)
        xT = x[rows].T.astype(base.dtype)  # [64, BPC]
        eT = comp_emb[rows].T.astype(base.dtype)
        pk[0:64, 0:BPC] = xT
        pk[64:128, 0:BPC] = xT
        pk[0:64, BPC : 2 * BPC] = eT
        pk[64:128, BPC : 2 * BPC] = eT
        in_maps.append(
            {
                f"pack{i}": np.ascontiguousarray(pk[:, c0:c1])
                for i, (c0, c1) in enumerate(zip(CHUNKS[:-1], CHUNKS[1:]))
            }
        )
    return in_maps


def kernel(**inputs):
    if "nc" not in _CACHE:
        _CACHE["nc"] = _build_nc()
    nc = _CACHE["nc"]
    from concourse.bass_utils import run_bass_kernel_spmd

    in_maps = _build_in_maps(inputs)
    res = run_bass_kernel_spmd(nc, in_maps, list(range(N_CORES)))
    outs = res.results
    price = np.concatenate([outs[c]["out"][0] for c in range(N_CORES)]).astype(np.float32)
    direction = np.concatenate([outs[c]["out"][1] for c in range(N_CORES)]).astype(np.float32)
    return price, direction


# revision 20
# speedup vs baseline: 1.0996x; 1.0996x over previous
"""Trainium2 Bass kernel for nn_EnhancedFinancialGAT.

Mathematical collapse: the reference broadcasts each batch item's feature
vector to all N=2000 graph nodes, so every node starts identical. A GAT
layer on identical node features returns, for every node, the attention-
weighted average of identical projected vectors -- and per-dst softmax
weights sum to exactly 1 in f32 (denom + 1e-16 == denom), so each layer
reduces to relu(h @ W.T + b). Every node stays identical through all 3
layers, and the company-node gather picks that shared vector. The whole
model is therefore an MLP:

  h = relu(x @ W_in.T + b_in)
  h = relu(h @ gat_W[l].T + gat_b[l])   for l in 0..2
  fused = relu(concat([h, emb[company_indices]]) @ W_fuse.T + b_fuse)
  price = W_p3 @ relu(W_p2 @ relu(W_p1 @ fused + b_p1) + b_p2) + b_p3
  direction = sigmoid(same with d-weights)

Verified numerically: collapsed-vs-full relative error ~2e-7 (pure f32
rounding noise of the softmax-weighted sums). The whole kernel is exact
f32: end-to-end relative error vs the reference is ~6e-7.

Sharding: data-parallel over batch (64 rows -> 8 rows/core). Weights are
replicated, pre-transposed on host into per-chunk contiguous DRAM
tensors (activations-transposed layout [feature, batch], so no on-device
transposes are needed anywhere).

Raw Bass program (no TileContext, no Block): everything lives in the
single entry basic block -- no branches (so no IRAM I$-miss stalls) and
no all-engine barrier waves beyond the framework's fixed preamble.
Loads are spread over the three DMA-capable engine queues (sync and
scalar are HWDGE, gpsimd is SWDGE) ordered by when their consumer needs
them, so the fp32 matmul chain starts as soon as chunk0 lands and rarely
waits on DMA again. The sigmoid ACT table set is preloaded via a dummy
activation while weights stream in. PSUM: 4 rotating [128,8] banks for
the wide groups (WAR is covered by the RAW waits since DVE is in-order)
+ 4 dedicated banks for the head tails = exactly 8 banks.
"""

import numpy as np

USE_F16 = False  # fp32 is exact (6e-7); f16 lands ~1e-3 (PE computes 16-bit at bf16 precision)

B = 64
N_CORES = 8
BPC = B // N_CORES  # batch rows per core

# -------- packed [128, COLS] layout (column offsets) --------
OFF_ACTS = 0                       # 16: xT 0:8, embT 8:16 (rows 0:64, dup to 64:128)
OFF_BIAS = 16                      # 16: see bias map below
OFF_WIN = 32                       # 128: rows 0:64 -> M[0:128], rows 64:128 -> M[128:256]
OFF_GAT = 176                      # 6 blocks of 256: block (l,k) at (2l+k)*256
OFF_FUSE = OFF_GAT + 6 * 256       # k0 [128,256], k1 [128,256]
OFF_FUSE2 = OFF_FUSE + 512         # k2 row-split, 128 cols
OFF_P1 = OFF_FUSE2 + 128           # k0 cols 0:128, k1 cols 128:256
OFF_D1 = OFF_P1 + 256
OFF_P2 = OFF_D1 + 256              # [128, 64]
OFF_D2 = OFF_P2 + 64
OFF_P3 = OFF_D2 + 64               # rows 0:64, 1 col
OFF_D3 = OFF_P3 + 1
COLS = OFF_D3 + 1                  # 2994

# bias columns (OFF_BIAS + j):
#  0,1: b_in | 2..7: gat_b (l,m) | 8,9: b_fuse | 10: b_p1 | 11: b_d1
#  12: b_p2 (rows 0:64) | 13: b_d2 | 14 row0: b_p3 | 15 row0: b_d3

# chunk boundaries: c0 = acts+bias+w_in, c1/c2/c3 = gat layers,
# c4 = fuse, c5 = heads
CHUNKS = [0, OFF_GAT, OFF_GAT + 512, OFF_GAT + 1024, OFF_FUSE, OFF_P1, COLS]

_CACHE = {}


def _build_nc():
    from contextlib import ExitStack

    import concourse.bass as bass
    import concourse.mybir as mybir

    f32 = mybir.dt.float32
    f16 = mybir.dt.float16 if USE_F16 else mybir.dt.float32
    ADD = mybir.AluOpType.add
    MAX = mybir.AluOpType.max

    nc = bass.Bass("TRN2", debug=False, num_devices=N_CORES)
    # one contiguous DRAM tensor per chunk: strided slices of a single
    # wide tensor DMA slowly; contiguous blocks run at queue line rate
    packs = [
        nc.declare_dram_parameter(f"pack{i}", [128, c1 - c0], f16, isOutput=False)
        for i, (c0, c1) in enumerate(zip(CHUNKS[:-1], CHUNKS[1:]))
    ]
    out_d = nc.declare_dram_parameter("out", [2, BPC], f32, isOutput=True)

    ctx = ExitStack()
    with ctx:
        sb = lambda nm, shape, dt: ctx.enter_context(nc.sbuf_tensor(nm, shape, dt))
        psb = lambda nm, shape: ctx.enter_context(nc.psum_tensor(nm, shape, f32))
        W = sb("W", [128, COLS], f16)
        h0 = [sb(f"h0_{i}", [128, BPC], f16) for i in range(2)]
        g1 = [sb(f"g1_{i}", [128, BPC], f16) for i in range(2)]
        g2 = [sb(f"g2_{i}", [128, BPC], f16) for i in range(2)]
        g3 = [sb(f"g3_{i}", [128, BPC], f16) for i in range(2)]
        fu = [sb(f"fu_{i}", [128, BPC], f16) for i in range(2)]
        a1p = sb("a1p", [128, BPC], f16)
        a1d = sb("a1d", [128, BPC], f16)
        a2p = sb("a2p", [64, BPC], f16)
        a2d = sb("a2d", [64, BPC], f16)
        price = sb("price", [1, BPC], f32)
        dirn = sb("dirn", [1, BPC], f32)
        scratch = sb("scratch", [1, BPC], f32)
        A = [psb(f"A{i}", [128, BPC]) for i in range(4)]
        Pp2 = psb("Pp2", [64, BPC])
        Pd2 = psb("Pd2", [64, BPC])
        Pp3 = psb("Pp3", [1, BPC])
        Pd3 = psb("Pd3", [1, BPC])

        csem = [ctx.enter_context(nc.semaphore(f"c{i}")) for i in range(6)]
        pe_sem = ctx.enter_context(nc.semaphore("pe"))
        dve_sem = ctx.enter_context(nc.semaphore("dve"))
        store_sem = ctx.enter_context(nc.semaphore("store"))
        all_sems = csem + [pe_sem, dve_sem, store_sem]

        # ---- loads: three parallel engine queues, ordered by need time.
        # Per-queue transfers serialize, so each stream is ordered by its
        # consumers: sync c0 (input layer) then gat1; scalar gat0 (+ ACT
        # table preload after) then fuse; gpsimd gat2 then heads.
        hoist_names = []

        def load(eng, i, hoist=False):
            c0, c1 = CHUNKS[i], CHUNKS[i + 1]
            m = eng.dma_start(out=W[:, c0:c1], in_=packs[i][:])
            m.then_inc(csem[i], 16)
            if hoist:
                hoist_names.append(m.ins.name)

        # Three parallel DMA streams, each ordered by consumer need time:
        # sync (HWDGE): c0 (input layer) then c2 (gat1); scalar (HWDGE):
        # c1 (gat0) then c4 (fuse); gpsimd (SWDGE): c3 (gat2) then c5
        # (heads). The four HWDGE loads are hoisted to the top of the
        # entry block (before the framework's startup barrier) so the
        # transfers run under the fixed ~5.5us preamble.
        load(nc.sync, 0, hoist=True)
        load(nc.scalar, 1, hoist=True)
        load(nc.sync, 2, hoist=True)
        # preload the sigmoid table set early (off the critical path)
        nc.scalar.activation(
            scratch[:], scratch[:], mybir.ActivationFunctionType.Sigmoid
        )
        load(nc.scalar, 4, hoist=True)
        load(nc.gpsimd, 3)
        load(nc.gpsimd, 5)

        def bias(j, r1=128, r0=0):
            return W[r0:r1, OFF_BIAS + j : OFF_BIAS + j + 1]

        # ---- PE program -------------------------------------------------
        pe = nc.tensor
        peng = nc.engines[mybir.EngineType.PE]

        def mm(out, lhsT, rhs, start, stop, inc=False):
            m = pe.matmul(out, lhsT, rhs, start=start, stop=stop)
            if inc:
                m.then_inc(pe_sem, 1)

        peng.wait_ge(csem[0], 16)
        for m in range(2):
            r = slice(64 * m, 64 * (m + 1))
            mm(A[m][:], W[r, OFF_WIN : OFF_WIN + 128], W[r, 0:BPC],
               True, True, inc=True)
        hh = h0
        for l in range(3):
            out = (g1, g2, g3)[l]
            c0 = OFF_GAT + (2 * l) * 256
            c1 = OFF_GAT + (2 * l + 1) * 256
            bank = [A[(2 + 2 * l) % 4], A[(3 + 2 * l) % 4]]
            peng.wait_ge(csem[1 + l], 16)
            peng.wait_ge(dve_sem, 2 * l + 1)          # hh[0] ready
            for m in range(2):
                mm(bank[m][:], W[:, c0 + m * 128 : c0 + m * 128 + 128],
                   hh[0][:], True, False)
            peng.wait_ge(dve_sem, 2 * l + 2)          # hh[1] ready
            for m in range(2):
                mm(bank[m][:], W[:, c1 + m * 128 : c1 + m * 128 + 128],
                   hh[1][:], False, True, inc=True)
            hh = out
        peng.wait_ge(csem[4], 16)
        peng.wait_ge(dve_sem, 7)
        for m in range(2):
            mm(A[m][:], W[:, OFF_FUSE + m * 128 : OFF_FUSE + m * 128 + 128],
               hh[0][:], True, False)
        peng.wait_ge(dve_sem, 8)
        for m in range(2):
            mm(A[m][:], W[:, OFF_FUSE + 256 + m * 128 : OFF_FUSE + 256 + m * 128 + 128],
               hh[1][:], False, False)
        for m in range(2):
            r = slice(64 * m, 64 * (m + 1))
            mm(A[m][:], W[r, OFF_FUSE2 : OFF_FUSE2 + 128], W[r, BPC : 2 * BPC],
               False, True, inc=True)
        peng.wait_ge(csem[5], 16)
        peng.wait_ge(dve_sem, 9)                      # fu[0]
        mm(A[2][:], W[:, OFF_P1 : OFF_P1 + 128], fu[0][:], True, False)
        mm(A[3][:], W[:, OFF_D1 : OFF_D1 + 128], fu[0][:], True, False)
        peng.wait_ge(dve_sem, 10)                     # fu[1]
        mm(A[2][:], W[:, OFF_P1 + 128 : OFF_P1 + 256], fu[1][:], False, True, inc=True)
        mm(A[3][:], W[:, OFF_D1 + 128 : OFF_D1 + 256], fu[1][:], False, True, inc=True)
        peng.wait_ge(dve_sem, 11)
        mm(Pp2[:], W[:, OFF_P2 : OFF_P2 + 64], a1p[:], True, True, inc=True)
        peng.wait_ge(dve_sem, 12)
        mm(Pd2[:], W[:, OFF_D2 : OFF_D2 + 64], a1d[:], True, True, inc=True)
        peng.wait_ge(dve_sem, 13)
        mm(Pp3[:], W[0:64, OFF_P3 : OFF_P3 + 1], a2p[:], True, True, inc=True)
        peng.wait_ge(dve_sem, 14)
        mm(Pd3[:], W[0:64, OFF_D3 : OFF_D3 + 1], a2d[:], True, True, inc=True)

        # ---- DVE program ------------------------------------------------
        veng = nc.engines[mybir.EngineType.DVE]

        def rb(out, psum, j, pe_need, r1=128):
            veng.wait_ge(pe_sem, pe_need)
            nc.vector.tensor_scalar(
                out, psum, bias(j, r1), 0.0, ADD, MAX
            ).then_inc(dve_sem, 1)

        rb(h0[0][:], A[0][:], 0, 1)
        rb(h0[1][:], A[1][:], 1, 2)
        for l, out in enumerate((g1, g2, g3)):
            for m in range(2):
                rb(out[m][:], A[(2 + 2 * l + m) % 4][:], 2 + 2 * l + m,
                   3 + 2 * l + m)
        rb(fu[0][:], A[0][:], 8, 9)
        rb(fu[1][:], A[1][:], 9, 10)
        rb(a1p[:], A[2][:], 10, 11)
        rb(a1d[:], A[3][:], 11, 12)
        rb(a2p[:], Pp2[:], 12, 13, r1=64)
        rb(a2d[:], Pd2[:], 13, 14, r1=64)
        veng.wait_ge(pe_sem, 15)
        nc.vector.tensor_scalar(
            price[:], Pp3[:], bias(14, 1), None, ADD
        ).then_inc(dve_sem, 1)

        # ---- ACT: real sigmoid + dir store (scalar is HWDGE-capable) ----
        aeng = nc.engines[mybir.EngineType.Activation]
        aeng.wait_ge(pe_sem, 16)
        nc.scalar.activation(
            dirn[:], Pd3[:], mybir.ActivationFunctionType.Sigmoid,
            bias=bias(15, 1),
        )
        nc.scalar.dma_start(out=out_d[1:2, :], in_=dirn[:]).then_inc(store_sem, 16)

        # ---- sync: price store ------------------------------------------
        seng = nc.engines[mybir.EngineType.SP]
        seng.wait_ge(dve_sem, 15)
        nc.sync.dma_start(out=out_d[0:1, :], in_=price[:]).then_inc(store_sem, 16)

        # ---- gpsimd: sole final waiter; clear sems for re-execution -----
        geng = nc.engines[mybir.EngineType.Pool]
        geng.wait_ge(store_sem, 32)
        nums = sorted(s.num for s in all_sems)
        lo, hi = nums[0], nums[-1]
        assert nums == list(range(lo, hi + 1)), nums
        nc.gpsimd.dma_reset(range(lo, hi + 1))
        nc.gpsimd.sem_clear(range(lo, hi + 1))

    # hoist the HWDGE loads ahead of the startup barrier: keep the
    # register/preamble-call setup first, then the loads, then the rest
    entry = nc.main_func.blocks[0]
    il = entry.instructions
    names = set(hoist_names)
    loads = [i for i in il if i.name in names]
    assert len(loads) == len(names), (len(loads), names)
    others = [i for i in il if i.name not in names]
    k = next(
        idx for idx, i in enumerate(others)
        if type(i).__name__ == "InstEventSemaphore"
    )
    il[:] = others[:k] + loads + others[k:]

    return nc


def _pack_host(inputs):
    f32 = lambda k: np.ascontiguousarray(np.asarray(inputs[k], dtype=np.float32))
    W_in, b_in = f32("W_in"), f32("b_in")
    gat_W, gat_b = f32("gat_W"), f32("gat_b")
    W_fuse, b_fuse = f32("W_fuse"), f32("b_fuse")
    W_p1, b_p1 = f32("W_p1"), f32("b_p1")
    W_p2, b_p2 = f32("W_p2"), f32("b_p2")
    W_p3, b_p3 = f32("W_p3"), f32("b_p3")
    W_d1, b_d1 = f32("W_d1"), f32("b_d1")
    W_d2, b_d2 = f32("W_d2"), f32("b_d2")
    W_d3, b_d3 = f32("W_d3"), f32("b_d3")

    np16 = np.float16 if USE_F16 else np.float32
    pk = np.zeros((128, COLS), np16)

    bias = pk[:, OFF_BIAS : OFF_BIAS + 16]
    bias[:, 0], bias[:, 1] = b_in[:128], b_in[128:]
    for l in range(3):
        for m in range(2):
            bias[:, 2 + 2 * l + m] = gat_b[l, 128 * m : 128 * (m + 1)]
    bias[:, 8], bias[:, 9] = b_fuse[:128], b_fuse[128:]
    bias[:, 10], bias[:, 11] = b_p1, b_d1
    bias[:64, 12], bias[:64, 13] = b_p2, b_d2
    bias[0, 14], bias[0, 15] = b_p3[0], b_d3[0]

    WinT = W_in.T.astype(np16)  # [64, 256]
    pk[0:64, OFF_WIN : OFF_WIN + 128] = WinT[:, 0:128]
    pk[64:128, OFF_WIN : OFF_WIN + 128] = WinT[:, 128:256]
    for l in range(3):
        GT = gat_W[l].T.astype(np16)  # [256, 256]
        for k in range(2):
            c = OFF_GAT + (2 * l + k) * 256
            pk[:, c : c + 256] = GT[128 * k : 128 * (k + 1), :]
    FT = W_fuse.T.astype(np16)  # [320, 256]
    pk[:, OFF_FUSE : OFF_FUSE + 256] = FT[0:128]
    pk[:, OFF_FUSE + 256 : OFF_FUSE + 512] = FT[128:256]
    pk[0:64, OFF_FUSE2 : OFF_FUSE2 + 128] = FT[256:320, 0:128]
    pk[64:128, OFF_FUSE2 : OFF_FUSE2 + 128] = FT[256:320, 128:256]
    for W1, off in ((W_p1, OFF_P1), (W_d1, OFF_D1)):
        T = W1.T.astype(np16)  # [256, 128]
        pk[:, off : off + 128] = T[0:128]
        pk[:, off + 128 : off + 256] = T[128:256]
    pk[:, OFF_P2 : OFF_P2 + 64] = W_p2.T.astype(np16)
    pk[:, OFF_D2 : OFF_D2 + 64] = W_d2.T.astype(np16)
    pk[0:64, OFF_P3] = W_p3[0].astype(np16)
    pk[0:64, OFF_D3] = W_d3[0].astype(np16)
    return pk


def _build_in_maps(inputs):
    x = np.asarray(inputs["x"], dtype=np.float32)
    ci = np.asarray(inputs["company_indices"]).astype(np.int64)
    emb = np.asarray(inputs["emb"], dtype=np.float32)
    comp_emb = emb[ci]  # [B, 64]

    base = _pack_host(inputs)
    in_maps = []
    for c in range(N_CORES):
        pk = base.copy()
        rows = slice(c * BPC, (c + 1) * BPC)
        xT = x[rows].T.astype(base.dtype)  # [64, BPC]
        eT = comp_emb[rows].T.astype(base.dtype)
        pk[0:64, 0:BPC] = xT
        pk[64:128, 0:BPC] = xT
        pk[0:64, BPC : 2 * BPC] = eT
        pk[64:128, BPC : 2 * BPC] = eT
        in_maps.append(
            {
                f"pack{i}": np.ascontiguousarray(pk[:, c0:c1])
                for i, (c0, c1) in enumerate(zip(CHUNKS[:-1], CHUNKS[1:]))
            }
        )
    return in_maps


def kernel(**inputs):
    if "nc" not in _CACHE:
        _CACHE["nc"] = _build_nc()
    nc = _CACHE["nc"]
    from concourse.bass_utils import run_bass_kernel_spmd

    in_maps = _build_in_maps(inputs)
    res = run_bass_kernel_spmd(nc, in_maps, list(range(N_CORES)))
    outs = res.results
    price = np.concatenate([outs[c]["out"][0] for c in range(N_CORES)]).astype(np.float32)
    direction = np.concatenate([outs[c]["out"][1] for c in range(N_CORES)]).astype(np.float32)
    return price, direction
